# revision 2
# baseline (speedup 1.0000x reference)
"""Trainium2 Bass kernel for CameraCorrector v2: camera-per-partition-slot layout.

Full inputs (N=4194304 pts, M=2048 cams) -> full [N,2] output.

Sharding: 256 cameras per core (8 cores). Each camera's points live in one
(partition, column-block) slot of 2048 columns; per-partition TS scalars
carry the camera params, so every DVE op runs wide (FD=2048) in a fast mode:
  products int16 TS @4x, merged adds int16 TT @2x, final muls @2x.
w is accumulated in offset int16 fixed point ((w-4.5)*4096), decoded inside
the Scalar-engine Reciprocal via its scale/bias: rw = 1/(4w) in fp16.
Device outputs (u/4, v/4) fp16; host rescales, scatters, and projects the
tiny spill set (points beyond 2048 per camera) in f64.
"""

import os
from contextlib import ExitStack

import numpy as np

N = 4_194_304
M = 2048
NCORES = 8
CAMS_PER_CORE = M // NCORES      # 256
F = 2048                         # slot length = columns per camera
NB = 2                           # batches (slot blocks) per core: 2*128 = 256 slots
WSCALE = 4096.0
WOFF = 4.5
USCALE = 4.0                     # device emits u/4 to stay inside fp16 range


# ----------------------------------------------------------------------------
# host-side math
# ----------------------------------------------------------------------------

def fold_table(intrinsics_noisy, R_noisy, t_noisy, intrinsic_deltas,
               rotation_deltas, translation_deltas):
    """Return tbl [M, 12] f64 folded homogeneous projection rows."""
    r = rotation_deltas.astype(np.float64)
    theta = np.linalg.norm(r, axis=-1, keepdims=True)
    k = r / np.maximum(theta, 1e-12)
    kx, ky, kz = k[:, 0], k[:, 1], k[:, 2]
    z = np.zeros_like(kx)
    K = np.stack([
        np.stack([z, -kz, ky], -1),
        np.stack([kz, z, -kx], -1),
        np.stack([-ky, kx, z], -1),
    ], axis=-2)
    st = np.sin(theta)[..., None]
    ct = np.cos(theta)[..., None]
    Rdelta = np.eye(3) + st * K + (1.0 - ct) * (K @ K)
    R = Rdelta @ R_noisy.astype(np.float64)
    t = (t_noisy + translation_deltas).astype(np.float64)
    Kc = (intrinsics_noisy + intrinsic_deltas).astype(np.float64)
    fx, fy, cx, cy = Kc[:, 0], Kc[:, 1], Kc[:, 2], Kc[:, 3]

    tbl = np.empty((M, 12), np.float64)
    for c in range(3):
        tbl[:, 0 + c] = fx * R[:, 0, c] + cx * R[:, 2, c]
        tbl[:, 3 + c] = fy * R[:, 1, c] + cy * R[:, 2, c]
        tbl[:, 6 + c] = R[:, 2, c]
    tbl[:, 9] = fx * t[:, 0] + cx * t[:, 2]
    tbl[:, 10] = fy * t[:, 1] + cy * t[:, 2]
    tbl[:, 11] = t[:, 2]
    return tbl


def host_project(X, cam, tbl64):
    """Reference-grade f64 projection for host-handled points."""
    A = tbl64[cam]
    Xd = X.astype(np.float64)
    nu = (A[:, 0:3] * Xd).sum(1) + A[:, 9]
    nv = (A[:, 3:6] * Xd).sum(1) + A[:, 10]
    w = (A[:, 6:9] * Xd).sum(1) + A[:, 11]
    return np.stack([nu / w, nv / w], -1).astype(np.float32)


# ----------------------------------------------------------------------------
# device kernel
# ----------------------------------------------------------------------------

def build_nc(num_devices=NCORES):
    import concourse.bass as bass
    import concourse.tile as tile
    from concourse import bacc, mybir

    f32 = mybir.dt.float32
    f16 = mybir.dt.float16
    i16 = mybir.dt.int16
    mult = mybir.AluOpType.mult
    add = mybir.AluOpType.add

    nc = bacc.Bacc(
        "TRN2",
        target_bir_lowering=False,
        debug=False,
        enable_asserts=False,
        num_devices=num_devices,
    )
    x_d = nc.dram_tensor("x", [NB * 3 * 128 * F], i16, kind="ExternalInput").ap()
    par_d = nc.dram_tensor("par", [128 * 12 * NB], f32, kind="ExternalInput").ap()
    uv_d = nc.dram_tensor("uv", [NB * 128 * 2 * F], f16, kind="ExternalOutput").ap()

    with tile.TileContext(nc) as tc, ExitStack() as ctx:
        par_pool = ctx.enter_context(tc.tile_pool(name="parp", bufs=1))
        x_pool = ctx.enter_context(tc.tile_pool(name="xp", bufs=3))
        p_pool = ctx.enter_context(tc.tile_pool(name="pp", bufs=3))
        w_pool = ctx.enter_context(tc.tile_pool(name="wp", bufs=3))
        uv_pool = ctx.enter_context(tc.tile_pool(name="uvp", bufs=3))

        par = par_pool.tile([128, 12 * NB], f32)
        nc.sync.dma_start(par[:], par_d.rearrange("(p a) -> p a", p=128))

        C = F // 2  # chain-op chunk columns (2 halves per slot block)
        for m in range(NB):
            xi = x_pool.tile([128, 3 * F], i16, tag="xi")
            src = x_d[m * 128 * 3 * F:(m + 1) * 128 * 3 * F]
            srcv = src.rearrange("(p a) -> p a", p=128)
            for c in range(3):
                nc.sync.dma_start(xi[:, c * F:(c + 1) * F], srcv[:, c * F:(c + 1) * F])

            def sc(i):
                return par[:, 12 * m + i:12 * m + i + 1]

            # full-F product planes: pa/pb/pc (fp16) nu|nv terms, qa/qb/qc (i16) w terms
            pa = p_pool.tile([128, 2 * F], f16, tag="pa")
            pb = p_pool.tile([128, 2 * F], f16, tag="pb")
            pc = p_pool.tile([128, 2 * F], f16, tag="pc")
            qa = p_pool.tile([128, F], i16, tag="qa")
            qb = p_pool.tile([128, F], i16, tag="qb")
            qc = p_pool.tile([128, F], i16, tag="qc")
            xs = [xi[:, c * F:(c + 1) * F] for c in range(3)]
            # DVE: 4 products (t-folds + w x0/x1)
            nc.vector.tensor_scalar(out=pa[:, 0:F], in0=xs[0], scalar1=sc(0),
                                    scalar2=sc(9), op0=mult, op1=add)
            nc.vector.tensor_scalar(out=pa[:, F:2 * F], in0=xs[0], scalar1=sc(3),
                                    scalar2=sc(10), op0=mult, op1=add)
            nc.vector.tensor_scalar(out=pb[:, 0:F], in0=xs[1], scalar1=sc(1),
                                    scalar2=None, op0=mult)
            nc.vector.tensor_scalar(out=pb[:, F:2 * F], in0=xs[1], scalar1=sc(4),
                                    scalar2=None, op0=mult)
            # ACT: 5 products (w row in f32, t2 folded into x0 term)
            nc.scalar.activation(out=qa[:], in_=xs[0],
                                 func=mybir.ActivationFunctionType.Identity,
                                 bias=sc(11), scale=sc(6))
            nc.scalar.activation(out=qb[:], in_=xs[1],
                                 func=mybir.ActivationFunctionType.Identity,
                                 bias=0.0, scale=sc(7))
            nc.scalar.activation(out=qc[:], in_=xs[2],
                                 func=mybir.ActivationFunctionType.Identity,
                                 bias=0.0, scale=sc(8))
            nc.scalar.activation(out=pc[:, 0:F], in_=xs[2],
                                 func=mybir.ActivationFunctionType.Identity,
                                 bias=0.0, scale=sc(2))
            nc.scalar.activation(out=pc[:, F:2 * F], in_=xs[2],
                                 func=mybir.ActivationFunctionType.Identity,
                                 bias=0.0, scale=sc(5))

            chunks = [(0, C), (C, C)] if m < NB - 1 else \
                     [(0, C), (C, C // 2), (C + C // 2, C // 2)]
            # pass 1: w chains + reciprocals for every chunk (drains ACT early)
            rws = []
            for (c0, cw) in chunks:
                cs = slice(c0, c0 + cw)
                nc.vector.tensor_tensor(out=qa[:, cs], in0=qa[:, cs], in1=qb[:, cs], op=add)
                wf = w_pool.tile([128, cw], f32, tag="wf")
                nc.vector.tensor_tensor(out=wf[:], in0=qa[:, cs], in1=qc[:, cs], op=add)
                rw = w_pool.tile([128, cw], f16, tag="rw")
                eng = nc.scalar
                eng.add_instruction(mybir.InstActivation(
                    name=nc.get_next_instruction_name(),
                    func=mybir.ActivationFunctionType.Reciprocal,
                    ins=[eng.lower_ap(wf[:]),
                         mybir.ImmediateValue(dtype=f32, value=USCALE * WOFF),
                         mybir.ImmediateValue(dtype=f32, value=USCALE / WSCALE),
                         mybir.ImmediateValue(dtype=f32, value=0.0)],
                    outs=[eng.lower_ap(rw[:])]))
                rws.append(rw)

            # pass 2: uv adds + muls + output, pure DVE
            for (c0, cw), rw in zip(chunks, rws):
                h2 = bass.AP(pa.tensor, pa[:].offset + c0,
                             [list(pa[:].ap[0]), [F, 2], [1, cw]])
                h2b = bass.AP(pb.tensor, pb[:].offset + c0,
                              [list(pb[:].ap[0]), [F, 2], [1, cw]])
                h2c = bass.AP(pc.tensor, pc[:].offset + c0,
                              [list(pc[:].ap[0]), [F, 2], [1, cw]])
                nc.vector.tensor_tensor(out=h2, in0=h2, in1=h2b, op=add)
                nc.vector.tensor_tensor(out=h2, in0=h2, in1=h2c, op=add)
                uv = uv_pool.tile([128, 2 * C], f16, tag="uv")
                uvv = uv[:, 0:2 * cw].rearrange("p (two f) -> p two f", two=2)
                rwrep = bass.AP(rw.tensor, rw[:].offset,
                                [list(rw[:].ap[0]), [0, 2], [1, cw]])
                nc.vector.tensor_tensor(out=uvv[:], in0=h2, in1=rwrep, op=mult)
                dst = uv_d[m * 128 * 2 * F:(m + 1) * 128 * 2 * F]
                dstv = dst.rearrange("(p two f) -> p two f", p=128, two=2)
                nc.sync.dma_start(dstv[:, :, c0:c0 + cw], uvv[:])

    nc.compile()
    return nc


def _install_ntff_shim():
    import sys
    import types
    try:
        from antenv.axon_hooks import get_axon_ntff_profile_hook  # noqa: F401
        return
    except ImportError:
        pass
    try:
        from trn_agent_boot.trn_boot import _ntff_profile_via_ctypes
        hook = _ntff_profile_via_ctypes("/opt/axon/libaxon_pjrt.so")
    except Exception:
        hook = None
    mod = types.ModuleType("antenv.axon_hooks")
    mod._hook = hook
    mod.get_axon_ntff_profile_hook = lambda: mod._hook
    mod.set_axon_ntff_profile_hook = lambda h: setattr(mod, "_hook", h)
    sys.modules["antenv.axon_hooks"] = mod
    import antenv
    antenv.axon_hooks = mod


_NC_CACHE = {}


def _get_nc():
    if "nc" not in _NC_CACHE:
        _NC_CACHE["nc"] = build_nc()
    return _NC_CACHE["nc"]


def host_prep(X_world, camera_indices, tbl64):
    """Sort points into per-camera slots; build per-core device inputs.

    Returns (in_maps, scatter_info, qscale).
    """
    qscale = 32600.0 / max(float(np.abs(X_world).max()), 1e-9)
    xq_all = np.rint(X_world.astype(np.float64) * qscale).astype(np.int16)

    # int16-saturation screen for the w row ((w-4.5)*4096 accumulation)
    toffmax = float(np.abs((tbl64[:, 11] - WOFF)).max()) * WSCALE
    xf = np.abs(xq_all).astype(np.float32)
    k = WSCALE / qscale
    nrm = np.sqrt((xf * xf).sum(1)) * k
    part01 = (xf[:, 0] + xf[:, 1]) * k
    bad = (nrm + toffmax > 31500.0) | (part01 + toffmax > 31500.0)

    order = np.argsort(camera_indices, kind="stable")
    counts = np.bincount(camera_indices, minlength=M)
    starts = np.zeros(M + 1, np.int64)
    np.cumsum(counts, out=starts[1:])
    cam_sorted = camera_indices[order]
    rank = np.arange(N, dtype=np.int64) - starts[cam_sorted]

    is_dev = (rank < F) & ~bad[order]
    spill_orig = order[~is_dev]
    spill_cam = cam_sorted[~is_dev]

    dev_orig = order[is_dev]          # original idx of each device point
    dev_cam = cam_sorted[is_dev]
    dev_rank = rank[is_dev]           # column within slot

    in_maps = []
    scatter = []
    for core in range(NCORES):
        lo, hi = core * CAMS_PER_CORE, (core + 1) * CAMS_PER_CORE
        msk = (dev_cam >= lo) & (dev_cam < hi)
        o = dev_orig[msk]
        slot = dev_cam[msk] - lo          # 0..255
        col = dev_rank[msk]
        mi = slot // 128                  # batch
        p = slot % 128                    # partition

        x_dev = np.zeros((NB, 128, 3, F), np.int16)
        for c in range(3):
            x_dev[mi, p, c, col] = xq_all[o, c]

        par = np.zeros((128, NB, 12), np.float64)
        cams = np.arange(lo, hi)
        t = tbl64[cams]                   # [256, 12]
        tt = t.reshape(NB, 128, 12).transpose(1, 0, 2)  # [128, NB, 12]
        par[:, :, 0:3] = tt[:, :, 0:3] / qscale          # nu coeffs
        par[:, :, 3:6] = tt[:, :, 3:6] / qscale          # nv coeffs
        par[:, :, 9] = tt[:, :, 9]                        # t0
        par[:, :, 10] = tt[:, :, 10]                      # t1
        par[:, :, 6:9] = tt[:, :, 6:9] / qscale * WSCALE  # w coeffs
        par[:, :, 11] = (tt[:, :, 11] - WOFF) * WSCALE    # (t2-WOFF)*WSCALE

        in_maps.append({
            "x": x_dev.reshape(-1),
            "par": par.astype(np.float32).reshape(128, -1).reshape(-1),
        })
        scatter.append((o, mi, p, col))
    return in_maps, scatter, spill_orig, spill_cam


def kernel(X_world, camera_indices, intrinsics_noisy, R_noisy, t_noisy,
           intrinsic_deltas, rotation_deltas, translation_deltas):
    from concourse.bass_utils import run_bass_kernel_spmd

    tbl64 = fold_table(intrinsics_noisy, R_noisy, t_noisy, intrinsic_deltas,
                       rotation_deltas, translation_deltas)
    in_maps, scatter, spill_orig, spill_cam = host_prep(
        X_world, camera_indices, tbl64)

    nc = _get_nc()
    trace = bool(int(os.environ.get("CAMCORR_TRACE", "0")))
    if trace:
        _install_ntff_shim()
    res = run_bass_kernel_spmd(nc, in_maps, core_ids=list(range(NCORES)),
                               trace=trace)
    if trace and res.exec_time_ns is not None:
        print(f"HW exec time: {res.exec_time_ns} ns")
        kernel.last_exec_time_ns = res.exec_time_ns

    out = np.empty((N, 2), np.float32)
    for core in range(NCORES):
        o, mi, p, col = scatter[core]
        uv_dev = res.results[core]["uv"].reshape(NB, 128, 2, F)
        out[o, 0] = uv_dev[mi, p, 0, col].astype(np.float32) * USCALE
        out[o, 1] = uv_dev[mi, p, 1, col].astype(np.float32) * USCALE
    if spill_orig.size:
        out[spill_orig] = host_project(X_world[spill_orig], spill_cam, tbl64)
    return out


kernel.last_exec_time_ns = None


# revision 3
# speedup vs baseline: 1.0749x; 1.0749x over previous
"""Trainium2 Bass kernel for CameraCorrector v2: camera-per-partition-slot layout.

Full inputs (N=4194304 pts, M=2048 cams) -> full [N,2] output.

Sharding: 256 cameras per core (8 cores). Each camera's points live in one
(partition, column-block) slot of 2048 columns; per-partition TS scalars
carry the camera params, so every DVE op runs wide (FD=2048) in a fast mode:
  products int16 TS @4x, merged adds int16 TT @2x, final muls @2x.
w is accumulated in offset int16 fixed point ((w-4.5)*4096), decoded inside
the Scalar-engine Reciprocal via its scale/bias: rw = 1/(4w) in fp16.
Device outputs (u/4, v/4) fp16; host rescales, scatters, and projects the
tiny spill set (points beyond 2048 per camera) in f64.
"""

import os
from contextlib import ExitStack

import numpy as np

N = 4_194_304
M = 2048
NCORES = 8
CAMS_PER_CORE = M // NCORES      # 256
F = 2048                         # slot length = columns per camera
NB = 2                           # batches (slot blocks) per core: 2*128 = 256 slots
WSCALE = 4096.0
WOFF = 4.5
USCALE = 4.0                     # device emits u/4 to stay inside fp16 range


# ----------------------------------------------------------------------------
# host-side math
# ----------------------------------------------------------------------------

def fold_table(intrinsics_noisy, R_noisy, t_noisy, intrinsic_deltas,
               rotation_deltas, translation_deltas):
    """Return tbl [M, 12] f64 folded homogeneous projection rows."""
    r = rotation_deltas.astype(np.float64)
    theta = np.linalg.norm(r, axis=-1, keepdims=True)
    k = r / np.maximum(theta, 1e-12)
    kx, ky, kz = k[:, 0], k[:, 1], k[:, 2]
    z = np.zeros_like(kx)
    K = np.stack([
        np.stack([z, -kz, ky], -1),
        np.stack([kz, z, -kx], -1),
        np.stack([-ky, kx, z], -1),
    ], axis=-2)
    st = np.sin(theta)[..., None]
    ct = np.cos(theta)[..., None]
    Rdelta = np.eye(3) + st * K + (1.0 - ct) * (K @ K)
    R = Rdelta @ R_noisy.astype(np.float64)
    t = (t_noisy + translation_deltas).astype(np.float64)
    Kc = (intrinsics_noisy + intrinsic_deltas).astype(np.float64)
    fx, fy, cx, cy = Kc[:, 0], Kc[:, 1], Kc[:, 2], Kc[:, 3]

    tbl = np.empty((M, 12), np.float64)
    for c in range(3):
        tbl[:, 0 + c] = fx * R[:, 0, c] + cx * R[:, 2, c]
        tbl[:, 3 + c] = fy * R[:, 1, c] + cy * R[:, 2, c]
        tbl[:, 6 + c] = R[:, 2, c]
    tbl[:, 9] = fx * t[:, 0] + cx * t[:, 2]
    tbl[:, 10] = fy * t[:, 1] + cy * t[:, 2]
    tbl[:, 11] = t[:, 2]
    return tbl


def host_project(X, cam, tbl64):
    """Reference-grade f64 projection for host-handled points."""
    A = tbl64[cam]
    Xd = X.astype(np.float64)
    nu = (A[:, 0:3] * Xd).sum(1) + A[:, 9]
    nv = (A[:, 3:6] * Xd).sum(1) + A[:, 10]
    w = (A[:, 6:9] * Xd).sum(1) + A[:, 11]
    return np.stack([nu / w, nv / w], -1).astype(np.float32)


# ----------------------------------------------------------------------------
# device kernel
# ----------------------------------------------------------------------------

def build_nc(num_devices=NCORES):
    import concourse.bass as bass
    import concourse.tile as tile
    from concourse import bacc, mybir

    f32 = mybir.dt.float32
    f16 = mybir.dt.float16
    i16 = mybir.dt.int16
    mult = mybir.AluOpType.mult
    add = mybir.AluOpType.add

    nc = bacc.Bacc(
        "TRN2",
        target_bir_lowering=False,
        debug=False,
        enable_asserts=False,
        num_devices=num_devices,
    )
    x_d = nc.dram_tensor("x", [NB * 3 * 128 * F], i16, kind="ExternalInput").ap()
    par_d = nc.dram_tensor("par", [128 * 12 * NB], f32, kind="ExternalInput").ap()
    uv_d = nc.dram_tensor("uv", [NB * 128 * 2 * F], f16, kind="ExternalOutput").ap()

    with tile.TileContext(nc) as tc, ExitStack() as ctx:
        par_pool = ctx.enter_context(tc.tile_pool(name="parp", bufs=1))
        x_pool = ctx.enter_context(tc.tile_pool(name="xp", bufs=3))
        p_pool = ctx.enter_context(tc.tile_pool(name="pp", bufs=3))
        w_pool = ctx.enter_context(tc.tile_pool(name="wp", bufs=3))
        uv_pool = ctx.enter_context(tc.tile_pool(name="uvp", bufs=3))

        par = par_pool.tile([128, 12 * NB], f32)
        nc.scalar.dma_start(par[:], par_d.rearrange("(p a) -> p a", p=128))

        C = F // 2  # chain-op chunk columns (2 halves per slot block)
        for m in range(NB):
            xi = x_pool.tile([128, 3 * F], i16, tag="xi")
            src = x_d[m * 128 * 3 * F:(m + 1) * 128 * 3 * F]
            srcv = src.rearrange("(p a) -> p a", p=128)
            for c in range(3):
                nc.sync.dma_start(xi[:, c * F:(c + 1) * F], srcv[:, c * F:(c + 1) * F])

            def sc(i):
                return par[:, 12 * m + i:12 * m + i + 1]

            # full-F product planes: pa/pb/pc (fp16) nu|nv terms, qa/qb/qc (i16) w terms
            pa = p_pool.tile([128, 2 * F], f16, tag="pa")
            pb = p_pool.tile([128, 2 * F], f16, tag="pb")
            pc = p_pool.tile([128, 2 * F], f16, tag="pc")
            qa = p_pool.tile([128, F], i16, tag="qa")
            qb = p_pool.tile([128, F], i16, tag="qb")
            qc = p_pool.tile([128, F], i16, tag="qc")
            xs = [xi[:, c * F:(c + 1) * F] for c in range(3)]
            # DVE: 4 products (t-folds + w x0/x1)
            nc.vector.tensor_scalar(out=pa[:, 0:F], in0=xs[0], scalar1=sc(0),
                                    scalar2=sc(9), op0=mult, op1=add)
            nc.vector.tensor_scalar(out=pa[:, F:2 * F], in0=xs[0], scalar1=sc(3),
                                    scalar2=sc(10), op0=mult, op1=add)
            nc.vector.tensor_scalar(out=pb[:, 0:F], in0=xs[1], scalar1=sc(1),
                                    scalar2=None, op0=mult)
            nc.vector.tensor_scalar(out=pb[:, F:2 * F], in0=xs[1], scalar1=sc(4),
                                    scalar2=None, op0=mult)
            # ACT: 5 products (w row in f32, t2 folded into x0 term)
            nc.scalar.activation(out=qa[:], in_=xs[0],
                                 func=mybir.ActivationFunctionType.Identity,
                                 bias=sc(11), scale=sc(6))
            nc.scalar.activation(out=qb[:], in_=xs[1],
                                 func=mybir.ActivationFunctionType.Identity,
                                 bias=0.0, scale=sc(7))
            nc.scalar.activation(out=qc[:], in_=xs[2],
                                 func=mybir.ActivationFunctionType.Identity,
                                 bias=0.0, scale=sc(8))
            nc.scalar.activation(out=pc[:, 0:F], in_=xs[2],
                                 func=mybir.ActivationFunctionType.Identity,
                                 bias=0.0, scale=sc(2))
            nc.scalar.activation(out=pc[:, F:2 * F], in_=xs[2],
                                 func=mybir.ActivationFunctionType.Identity,
                                 bias=0.0, scale=sc(5))

            chunks = [(0, C), (C, C)] if m < NB - 1 else \
                     [(0, C), (C, C // 2), (C + C // 2, C // 2)]
            # pass 1: w chains + reciprocals for every chunk (drains ACT early)
            rws = []
            for (c0, cw) in chunks:
                cs = slice(c0, c0 + cw)
                nc.vector.tensor_tensor(out=qa[:, cs], in0=qa[:, cs], in1=qb[:, cs], op=add)
                wf = w_pool.tile([128, cw], f32, tag="wf")
                nc.vector.tensor_tensor(out=wf[:], in0=qa[:, cs], in1=qc[:, cs], op=add)
                rw = w_pool.tile([128, cw], f16, tag="rw")
                eng = nc.scalar
                eng.add_instruction(mybir.InstActivation(
                    name=nc.get_next_instruction_name(),
                    func=mybir.ActivationFunctionType.Reciprocal,
                    ins=[eng.lower_ap(wf[:]),
                         mybir.ImmediateValue(dtype=f32, value=USCALE * WOFF),
                         mybir.ImmediateValue(dtype=f32, value=USCALE / WSCALE),
                         mybir.ImmediateValue(dtype=f32, value=0.0)],
                    outs=[eng.lower_ap(rw[:])]))
                rws.append(rw)

            # pass 2: uv adds + muls + output, pure DVE
            for (c0, cw), rw in zip(chunks, rws):
                h2 = bass.AP(pa.tensor, pa[:].offset + c0,
                             [list(pa[:].ap[0]), [F, 2], [1, cw]])
                h2b = bass.AP(pb.tensor, pb[:].offset + c0,
                              [list(pb[:].ap[0]), [F, 2], [1, cw]])
                h2c = bass.AP(pc.tensor, pc[:].offset + c0,
                              [list(pc[:].ap[0]), [F, 2], [1, cw]])
                nc.vector.tensor_tensor(out=h2, in0=h2, in1=h2b, op=add)
                nc.vector.tensor_tensor(out=h2, in0=h2, in1=h2c, op=add)
                uv = uv_pool.tile([128, 2 * C], f16, tag="uv")
                uvv = uv[:, 0:2 * cw].rearrange("p (two f) -> p two f", two=2)
                rwrep = bass.AP(rw.tensor, rw[:].offset,
                                [list(rw[:].ap[0]), [0, 2], [1, cw]])
                nc.vector.tensor_tensor(out=uvv[:], in0=h2, in1=rwrep, op=mult)
                dst = uv_d[m * 128 * 2 * F:(m + 1) * 128 * 2 * F]
                dstv = dst.rearrange("(p two f) -> p two f", p=128, two=2)
                nc.sync.dma_start(dstv[:, :, c0:c0 + cw], uvv[:])

    nc.compile()
    return nc


def _install_ntff_shim():
    import sys
    import types
    try:
        from antenv.axon_hooks import get_axon_ntff_profile_hook  # noqa: F401
        return
    except ImportError:
        pass
    try:
        from trn_agent_boot.trn_boot import _ntff_profile_via_ctypes
        hook = _ntff_profile_via_ctypes("/opt/axon/libaxon_pjrt.so")
    except Exception:
        hook = None
    mod = types.ModuleType("antenv.axon_hooks")
    mod._hook = hook
    mod.get_axon_ntff_profile_hook = lambda: mod._hook
    mod.set_axon_ntff_profile_hook = lambda h: setattr(mod, "_hook", h)
    sys.modules["antenv.axon_hooks"] = mod
    import antenv
    antenv.axon_hooks = mod


_NC_CACHE = {}


def _get_nc():
    if "nc" not in _NC_CACHE:
        _NC_CACHE["nc"] = build_nc()
    return _NC_CACHE["nc"]


def host_prep(X_world, camera_indices, tbl64):
    """Sort points into per-camera slots; build per-core device inputs.

    Returns (in_maps, scatter_info, qscale).
    """
    qscale = 32600.0 / max(float(np.abs(X_world).max()), 1e-9)
    xq_all = np.rint(X_world.astype(np.float64) * qscale).astype(np.int16)

    # int16-saturation screen for the w row ((w-4.5)*4096 accumulation)
    toffmax = float(np.abs((tbl64[:, 11] - WOFF)).max()) * WSCALE
    xf = np.abs(xq_all).astype(np.float32)
    k = WSCALE / qscale
    nrm = np.sqrt((xf * xf).sum(1)) * k
    part01 = (xf[:, 0] + xf[:, 1]) * k
    bad = (nrm + toffmax > 31500.0) | (part01 + toffmax > 31500.0)

    order = np.argsort(camera_indices, kind="stable")
    counts = np.bincount(camera_indices, minlength=M)
    starts = np.zeros(M + 1, np.int64)
    np.cumsum(counts, out=starts[1:])
    cam_sorted = camera_indices[order]
    rank = np.arange(N, dtype=np.int64) - starts[cam_sorted]

    is_dev = (rank < F) & ~bad[order]
    spill_orig = order[~is_dev]
    spill_cam = cam_sorted[~is_dev]

    dev_orig = order[is_dev]          # original idx of each device point
    dev_cam = cam_sorted[is_dev]
    dev_rank = rank[is_dev]           # column within slot

    in_maps = []
    scatter = []
    for core in range(NCORES):
        lo, hi = core * CAMS_PER_CORE, (core + 1) * CAMS_PER_CORE
        msk = (dev_cam >= lo) & (dev_cam < hi)
        o = dev_orig[msk]
        slot = dev_cam[msk] - lo          # 0..255
        col = dev_rank[msk]
        mi = slot // 128                  # batch
        p = slot % 128                    # partition

        x_dev = np.zeros((NB, 128, 3, F), np.int16)
        for c in range(3):
            x_dev[mi, p, c, col] = xq_all[o, c]

        par = np.zeros((128, NB, 12), np.float64)
        cams = np.arange(lo, hi)
        t = tbl64[cams]                   # [256, 12]
        tt = t.reshape(NB, 128, 12).transpose(1, 0, 2)  # [128, NB, 12]
        par[:, :, 0:3] = tt[:, :, 0:3] / qscale          # nu coeffs
        par[:, :, 3:6] = tt[:, :, 3:6] / qscale          # nv coeffs
        par[:, :, 9] = tt[:, :, 9]                        # t0
        par[:, :, 10] = tt[:, :, 10]                      # t1
        par[:, :, 6:9] = tt[:, :, 6:9] / qscale * WSCALE  # w coeffs
        par[:, :, 11] = (tt[:, :, 11] - WOFF) * WSCALE    # (t2-WOFF)*WSCALE

        in_maps.append({
            "x": x_dev.reshape(-1),
            "par": par.astype(np.float32).reshape(128, -1).reshape(-1),
        })
        scatter.append((o, mi, p, col))
    return in_maps, scatter, spill_orig, spill_cam


def kernel(X_world, camera_indices, intrinsics_noisy, R_noisy, t_noisy,
           intrinsic_deltas, rotation_deltas, translation_deltas):
    from concourse.bass_utils import run_bass_kernel_spmd

    tbl64 = fold_table(intrinsics_noisy, R_noisy, t_noisy, intrinsic_deltas,
                       rotation_deltas, translation_deltas)
    in_maps, scatter, spill_orig, spill_cam = host_prep(
        X_world, camera_indices, tbl64)

    nc = _get_nc()
    trace = bool(int(os.environ.get("CAMCORR_TRACE", "0")))
    if trace:
        _install_ntff_shim()
    res = run_bass_kernel_spmd(nc, in_maps, core_ids=list(range(NCORES)),
                               trace=trace)
    if trace and res.exec_time_ns is not None:
        print(f"HW exec time: {res.exec_time_ns} ns")
        kernel.last_exec_time_ns = res.exec_time_ns

    out = np.empty((N, 2), np.float32)
    for core in range(NCORES):
        o, mi, p, col = scatter[core]
        uv_dev = res.results[core]["uv"].reshape(NB, 128, 2, F)
        out[o, 0] = uv_dev[mi, p, 0, col].astype(np.float32) * USCALE
        out[o, 1] = uv_dev[mi, p, 1, col].astype(np.float32) * USCALE
    if spill_orig.size:
        out[spill_orig] = host_project(X_world[spill_orig], spill_cam, tbl64)
    return out


kernel.last_exec_time_ns = None
